# revision 17
# baseline (speedup 1.0000x reference)
"""MoE SwiGLU FFN (grouped GEMM) Trainium2 kernel.

Problem: E=32 experts, T=65536 tokens pre-sorted by expert (uniform 2048
tokens/expert), D=512, H=1024.
    h1 = ragged_dot(x, w1) + b1[seg]; h3 = ragged_dot(x, w3) + b3[seg]
    out = ragged_dot(silu(h1)*h3, w2) + b2[seg]

Sharding: expert parallelism across 8 cores. Tokens are pre-sorted and
uniformly dispatched, so expert-parallel == token-parallel: core c owns
experts [4c, 4c+4) and token rows [8192c, 8192(c+1)). No collectives.

Per-core kernel (all matmuls float32r with fp32 PSUM accumulation;
fp32r streams 1 col/cycle; measured in-kernel MM spacing 233 ns vs 259
for bf16 — bf16's FWL weight loads contend with the rhs stream, so
fp32r is the FASTER operand dtype here, with ~16x lower error):
  - x slab is passed host-transposed as xt [D, TPC] so every DMA is a
    contiguous-row load (the contraction over D needs D on partitions).
  - weight/x tiles are float32r; DRAM fp32 is bitcast on load (the PE
    rounds internally; measured identical error to explicit cast-DMA).
  - b2 is host-broadcast to [EPC, 128, D] bf16 and DMA'd straight into
    SBUF (a PE ones-row broadcast would run as a true-fp32 LOW_HIGH
    2-pass matmul costing ~2.3us PE per expert).
  - GEMM1/3 produce H^T tiles [H-part, token-free]; ACT applies
    silu(psum1+b1) (bias is per-partition in this layout), DVE fuses
    (psum3+b3)*silu -> fp32r in one scalar_tensor_tensor op; GEMM2 then
    contracts H on partitions with no further transposes, and DVE evicts
    psum2 + b2tile -> fp32 out tile, stored contiguously.
  - chunk-level software pipeline: GEMM2 of chunk i is emitted after
    GEMM1/3 of chunk i+1 so PE never waits on the SwiGLU tail.
  - prologue: expert 0 loads are ordered w1 -> x(chunk0) -> w3 (x on the
    sync-engine HW DGE so it doesn't queue behind weight descriptor
    builds) so the first GEMM1 m-tiles can start as early as possible.
  - _prune_tick_incs removes the per-matmul Tile tick-semaphore
    increments nobody waits on (~14-17 ns NX overhead per matmul).
"""

import numpy as np

import concourse.bass as bass
import concourse.mybir as mybir
from concourse.bass_utils import run_bass_kernel_spmd
from concourse.tile import TileContext

E, T, D, H = 32, 65536, 512, 1024
NCORES = 8
EPC = E // NCORES        # experts per core
TPC = T // NCORES        # tokens per core
TPE = T // E             # tokens per expert
NT = 512                 # token chunk (one PSUM bank in fp32)
P = 128

FP32 = mybir.dt.float32
BF16 = mybir.dt.bfloat16
# matmul operand dtype: float32r = PE's rounded-fp32 mode — 1 cycle/row like
# bf16 but without bf16's FWL/xbus contention (233 vs 259 ns/MM measured
# in-kernel) and ~28x lower error (1.5e-4 vs 4.1e-3).
DT_MM = mybir.dt.float32r
NP_BF16 = mybir.dt.np(BF16)
AF = mybir.ActivationFunctionType
ALU = mybir.AluOpType


def _split_sync_waits(nc, max_waits=1):
    """The external neuronxcc walrus only accepts one sync-wait command per
    instruction; hoist excess waits onto preceding NoOps on the same engine."""
    n = 0
    for fn in nc.m.functions:
        for bb in fn.blocks:
            insts = bb.instructions
            i = 0
            while i < len(insts):
                inst = insts[i]
                si = inst.sync_info
                if si is not None and len(si.on_wait) > max_waits:
                    waits = list(si.on_wait)
                    while len(waits) > max_waits:
                        chunk, waits = waits[:max_waits], waits[max_waits:]
                        nop = mybir.InstNoOp(name=f"wait-split-{n}", ins=[], outs=[])
                        n += 1
                        nop.engine = inst.engine
                        nop.sync_info = mybir.SyncInfo(on_wait=chunk, on_update=[])
                        insts.insert(i, nop)
                        i += 1
                    inst.sync_info = mybir.SyncInfo(on_wait=waits, on_update=si.on_update)
                i += 1
    return n


def _prune_tick_incs(nc, engine=mybir.EngineType.PE, prefix="PE_"):
    """Drop Tile tick-semaphore increments on `engine` whose cumulative value
    is never referenced by any wait, renumbering the remaining thresholds.
    Each on_update costs ~14-17 ns of engine NX time (EVT register write), so
    a matmul-dense engine pays it on every instruction."""
    # 1. find the tick sem: updated sem-inc by instructions on `engine`
    sem_ids = set()
    for fn in nc.m.functions:
        for bb in fn.blocks:
            for inst in bb.instructions:
                si = inst.sync_info
                if si is None or inst.engine != engine:
                    continue
                for u in si.on_update:
                    if (getattr(u, "ant_name", "") or "").startswith(prefix):
                        sem_ids.add(u.id)
    if len(sem_ids) != 1:
        return 0
    sem = sem_ids.pop()

    # 2. collect every wait on that sem (any engine, any block)
    waits = []
    for fn in nc.m.functions:
        for bb in fn.blocks:
            for inst in bb.instructions:
                si = inst.sync_info
                if si is None:
                    continue
                for w in si.on_wait:
                    if w.id == sem:
                        if w.wait_mode != "sem-ge-imm" or w.wait_reg is not None:
                            return 0  # unexpected wait form; bail out
                        waits.append(w)

    # 3. walk increments in program order (blocks are linear here)
    incs = []  # (inst, cumulative_value)
    cum = 0
    for fn in nc.m.functions:
        for bb in fn.blocks:
            for inst in bb.instructions:
                si = inst.sync_info
                if si is None or inst.engine != engine:
                    continue
                for u in si.on_update:
                    if u.id == sem:
                        if u.update_mode != "sem-inc" or u.update_value != 1:
                            return 0
                        cum += 1
                        incs.append((inst, cum))

    needed = sorted({w.wait_value for w in waits if w.wait_value > 0})
    if needed and needed[-1] > cum:
        return 0  # wait beyond total — leave untouched
    keep = set(needed)
    if incs:
        keep.add(incs[-1][1])  # keep final total for drains/debug
    rank = {v: i + 1 for i, v in enumerate(sorted(keep))}

    removed = 0
    for inst, c in incs:
        if c not in keep:
            si = inst.sync_info
            inst.sync_info = mybir.SyncInfo(
                on_wait=si.on_wait,
                on_update=[u for u in si.on_update if u.id != sem])
            removed += 1
    for w in waits:
        if w.wait_value > 0:
            w.wait_value = rank[w.wait_value]
    return removed


def build_nc():
    nc = bass.Bass()

    xt = nc.declare_dram_parameter("xt", [D, TPC], FP32, isOutput=False)
    w1 = nc.declare_dram_parameter("w1", [EPC, D, H], FP32, isOutput=False)
    # b1/b3 come in pre-transposed [P, H/P]: the natural (m p) -> p m
    # rearrange load lowers to 1024 four-byte DMA descriptors, a packet
    # storm that clogs the queue for several us.
    b1 = nc.declare_dram_parameter("b1", [EPC, P, H // P], FP32, isOutput=False)
    w3 = nc.declare_dram_parameter("w3", [EPC, D, H], FP32, isOutput=False)
    b3 = nc.declare_dram_parameter("b3", [EPC, P, H // P], FP32, isOutput=False)
    w2 = nc.declare_dram_parameter("w2", [EPC, H, D], FP32, isOutput=False)
    b2 = nc.declare_dram_parameter("b2", [EPC, P, D], BF16, isOutput=False)
    # bf16 copies of expert-0's GEMM1/3 operands: half the DMA bytes, so the
    # first chunk's matmuls start ~7us earlier; only chunk (0,0) uses them
    # (1/16 of rows at bf16-grade error, total rel-l2 stays ~1e-3).
    w1b = nc.declare_dram_parameter("w1b", [D, H], BF16, isOutput=False)
    w3b = nc.declare_dram_parameter("w3b", [D, H], BF16, isOutput=False)
    xb = nc.declare_dram_parameter("xb", [D, NT], BF16, isOutput=False)
    out = nc.declare_dram_parameter("out", [TPC, D], FP32, isOutput=True)

    KD = D // P              # 4 k-tiles for GEMM1/3
    KH = H // P              # 8 k-tiles for GEMM2
    MH = H // P              # 8 h m-tiles per chunk
    NCHUNK = TPE // NT       # 4 chunks per expert
    MT = NT // P             # 4 token sub-tiles per chunk

    with TileContext(nc) as tc:
        with (
            tc.tile_pool(name="w1p", bufs=2 * KD) as w1pool,
            tc.tile_pool(name="w3p", bufs=2 * KD) as w3pool,
            tc.tile_pool(name="w2p", bufs=2 * KH) as w2pool,
            tc.tile_pool(name="bias", bufs=2) as bias_pool,
            tc.tile_pool(name="xp", bufs=3 * KD) as xpool,
            tc.tile_pool(name="hp", bufs=2 * MH) as hpool,
            tc.tile_pool(name="t1p", bufs=4) as t1pool,
            tc.tile_pool(name="op", bufs=4) as opool,
            # ps13 confined to PSUM banks 0-3 and ps2 to 4-6: consecutive-MM
            # bank transitions that cross the 0-3/4-7 bank-group boundary
            # slow the MM stream (measured 227 same-bank vs 233+ crossing).
            tc.tile_pool(name="ps13", bufs=4, space="PSUM") as ps13,
            tc.tile_pool(name="ps2", bufs=4, space="PSUM") as ps2,
        ):
            def load_w(e, which, wpool, wt):
                tiles = []
                for k in range(KD if which != "w2" else KH):
                    ncols = H if which != "w2" else D
                    t = wpool.tile([P, ncols], DT_MM, tag=which,
                                   name=f"{which}_{e}_{k}")
                    nc.gpsimd.dma_start(
                        out=t[:], in_=wt[e, k * P:(k + 1) * P, :].bitcast(DT_MM))
                    tiles.append(t)
                return tiles

            def load_b13(e):
                b1s = bias_pool.tile([P, MH], FP32, tag="b1", name=f"b1_{e}")
                nc.sync.dma_start(out=b1s[:], in_=b1[e])
                b3s = bias_pool.tile([P, MH], FP32, tag="b3", name=f"b3_{e}")
                nc.sync.dma_start(out=b3s[:], in_=b3[e])
                return b1s, b3s

            def load_x(e, c, eng):
                t0 = e * TPE + c * NT
                xbf = []
                for k in range(KD):
                    t = xpool.tile([P, NT], DT_MM, tag="x", name=f"x_{e}_{c}_{k}")
                    eng.dma_start(
                        out=t[:], in_=xt[k * P:(k + 1) * P, t0:t0 + NT].bitcast(DT_MM))
                    xbf.append(t)
                return xbf

            def load_w2b2(e, wts):
                # w2/b2 stay on gpsimd: issuing them from ACT deadlocks (the
                # w2-slot-free wait blocks ACT's strict FIFO, but freeing the
                # slot needs GEMM2 -> h tiles -> DVE stt -> ACT silu).
                wts["w2"] = load_w(e, "w2", w2pool, w2)
                b2b = bias_pool.tile([P, D], BF16, tag="b2b", name=f"b2b_{e}")
                nc.sync.dma_start(out=b2b[:], in_=b2[e])
                wts["b2"] = b2b

            def emit_gemm13(e, c, wts, xbf):
                """GEMM1+GEMM3+SwiGLU for chunk c of expert e -> 8 H^T tiles."""
                htiles = []
                for m in range(MH):
                    p1 = ps13.tile([P, NT], FP32, tag="p13", name=f"p1_{e}_{c}_{m}")
                    for k in range(KD):
                        nc.tensor.matmul(
                            p1[:], lhsT=wts["w1"][k][:, m * P:(m + 1) * P], rhs=xbf[k][:],
                            start=(k == 0), stop=(k == KD - 1))
                    p3 = ps13.tile([P, NT], FP32, tag="p13", name=f"p3_{e}_{c}_{m}")
                    for k in range(KD):
                        nc.tensor.matmul(
                            p3[:], lhsT=wts["w3"][k][:, m * P:(m + 1) * P], rhs=xbf[k][:],
                            start=(k == 0), stop=(k == KD - 1))
                    t1 = t1pool.tile([P, NT], FP32, tag="t1", name=f"t1_{e}_{c}_{m}")
                    nc.scalar.activation(t1[:], p1[:], AF.Silu,
                                         bias=wts["b1"][:, m:m + 1], scale=1.0)
                    hbf = hpool.tile([P, NT], DT_MM, tag="h", name=f"h_{e}_{c}_{m}")
                    nc.vector.scalar_tensor_tensor(
                        out=hbf[:], in0=p3[:], scalar=wts["b3"][:, m:m + 1], in1=t1[:],
                        op0=ALU.add, op1=ALU.mult)
                    htiles.append(hbf)
                return htiles

            def emit_gemm2(e, c, wts, htiles, split=False):
                t0 = e * TPE + c * NT
                if not split:
                    for mt in range(MT):
                        p2 = ps2.tile([P, D], FP32, tag="p2", name=f"p2_{e}_{c}_{mt}")
                        for k in range(KH):
                            nc.tensor.matmul(
                                p2[:], lhsT=htiles[k][:, mt * P:(mt + 1) * P],
                                rhs=wts["w2"][k][:],
                                start=(k == 0), stop=(k == KH - 1))
                        ot = opool.tile([P, D], FP32, tag="o", name=f"o_{e}_{c}_{mt}")
                        nc.vector.tensor_add(ot[:], p2[:], wts["b2"][:])
                        nc.sync.dma_start(
                            out=out[t0 + mt * P:t0 + (mt + 1) * P, :], in_=ot[:])
                    return
                # last chunk: no next GEMM1/3 to hide the SwiGLU tail behind,
                # so run k=0..3 for all mt first (only needs h0-3, ready well
                # before h7) and finish k=4..7 when the last h tiles land.
                p2s = [ps2.tile([P, D], FP32, tag="p2", name=f"p2_{e}_{c}_{mt}")
                       for mt in range(MT)]
                for mt in range(MT):
                    for k in range(KH // 2):
                        nc.tensor.matmul(
                            p2s[mt][:], lhsT=htiles[k][:, mt * P:(mt + 1) * P],
                            rhs=wts["w2"][k][:],
                            start=(k == 0), stop=False, skip_group_check=True)
                for mt in range(MT):
                    for k in range(KH // 2, KH):
                        nc.tensor.matmul(
                            p2s[mt][:], lhsT=htiles[k][:, mt * P:(mt + 1) * P],
                            rhs=wts["w2"][k][:],
                            start=False, stop=(k == KH - 1), skip_group_check=True)
                    ot = opool.tile([P, D], FP32, tag="o", name=f"o_{e}_{c}_{mt}")
                    nc.vector.tensor_add(ot[:], p2s[mt][:], wts["b2"][:])
                    nc.sync.dma_start(
                        out=out[t0 + mt * P:t0 + (mt + 1) * P, :], in_=ot[:])

            def load_e0_bf16():
                """bf16 copies of (w1, w3, x-chunk-0) for expert 0: 2.25MB
                instead of 5MB gates the first GEMM1/3 chains."""
                w1t, w3t, xbt = [], [], []
                for k in range(KD):
                    t = w1pool.tile([P, H], BF16, tag="w1b", bufs=KD,
                                    name=f"w1b_{k}")
                    nc.gpsimd.dma_start(out=t[:], in_=w1b[k * P:(k + 1) * P, :])
                    w1t.append(t)
                for k in range(KD):
                    t = xpool.tile([P, NT], BF16, tag="xb", bufs=KD,
                                   name=f"xb_{k}")
                    nc.sync.dma_start(out=t[:], in_=xb[k * P:(k + 1) * P, :])
                    xbt.append(t)
                for k in range(KD):
                    t = w3pool.tile([P, H], BF16, tag="w3b", bufs=KD,
                                    name=f"w3b_{k}")
                    nc.gpsimd.dma_start(out=t[:], in_=w3b[k * P:(k + 1) * P, :])
                    w3t.append(t)
                return w1t, w3t, xbt

            # chunk-level pipeline across the whole (expert, chunk) sequence
            pending = None  # (e, c, wts, htiles, split)
            for e in range(EPC):
                # DMA order matters for the prologue: biases first (tiny, and
                # chunk-0's SILU gates PSUM recycling on b1), then w1 (gates
                # the first p1 m-tile), chunk-0 x on the sync HW DGE (runs
                # concurrently with gpsimd weight loads), then w3.
                wts_e = dict()
                wts_e["b1"], wts_e["b3"] = load_b13(e)
                if e == 0:
                    w1t, w3t, xbt = load_e0_bf16()
                    wts0 = dict(wts_e, w1=w1t, w3=w3t)
                    x0 = xbt
                    wts_e["w1"] = load_w(e, "w1", w1pool, w1)
                    wts_e["w3"] = load_w(e, "w3", w3pool, w3)
                else:
                    wts_e["w1"] = load_w(e, "w1", w1pool, w1)
                    x0 = load_x(e, 0, nc.gpsimd)
                    wts_e["w3"] = load_w(e, "w3", w3pool, w3)
                for c in range(NCHUNK):
                    xbf = x0 if c == 0 else load_x(e, c, nc.gpsimd)
                    h = emit_gemm13(e, c, wts0 if (e == 0 and c == 0) else wts_e,
                                    xbf)
                    if c == 0:
                        load_w2b2(e, wts_e)
                    if pending is not None:
                        emit_gemm2(*pending)
                    pending = (e, c, wts_e, h)
            emit_gemm2(*pending, split=True)

    _prune_tick_incs(nc)
    _split_sync_waits(nc)
    return nc


_NC_CACHE = {}


def _get_nc():
    if "nc" not in _NC_CACHE:
        _NC_CACHE["nc"] = build_nc()
    return _NC_CACHE["nc"]


def prep_in_maps(x, w1, b1, w3, b3, w2, b2):
    x = np.asarray(x, np.float32)
    b2f = np.asarray(b2, np.float32).astype(NP_BF16)
    in_maps = []
    for c in range(NCORES):
        es = slice(c * EPC, (c + 1) * EPC)
        xt_c = np.ascontiguousarray(x[c * TPC:(c + 1) * TPC].T)
        w1_c = np.ascontiguousarray(w1[es], dtype=np.float32)
        w3_c = np.ascontiguousarray(w3[es], dtype=np.float32)
        in_maps.append(dict(
            xt=xt_c,
            w1=w1_c,
            b1=np.ascontiguousarray(
                np.asarray(b1[es], np.float32).reshape(EPC, H // P, P)
                .transpose(0, 2, 1)),
            w3=w3_c,
            b3=np.ascontiguousarray(
                np.asarray(b3[es], np.float32).reshape(EPC, H // P, P)
                .transpose(0, 2, 1)),
            w2=np.ascontiguousarray(w2[es], dtype=np.float32),
            b2=np.ascontiguousarray(
                np.broadcast_to(b2f[es][:, None, :], (EPC, P, D))),
            w1b=np.ascontiguousarray(w1_c[0].astype(NP_BF16)),
            w3b=np.ascontiguousarray(w3_c[0].astype(NP_BF16)),
            xb=np.ascontiguousarray(xt_c[:, 0:NT].astype(NP_BF16)),
        ))
    return in_maps


def _kernel_np_fallback(x, w1, b1, w3, b3, w2, b2, group_sizes):
    """Numpy reference path for non-uniform group sizes (not expected)."""
    bounds = np.cumsum(group_sizes)
    seg = np.searchsorted(bounds, np.arange(x.shape[0]), side="right")
    out = np.empty((x.shape[0], w2.shape[2]), np.float32)
    start = 0
    for e in range(len(group_sizes)):
        stop = start + int(group_sizes[e])
        xs = x[start:stop]
        h1 = xs @ w1[e] + b1[e]
        h3 = xs @ w3[e] + b3[e]
        h = (h1 / (1.0 + np.exp(-h1))) * h3
        out[start:stop] = h @ w2[e] + b2[e]
        start = stop
    return out


def kernel(x, w1, b1, w3, b3, w2, b2, group_sizes):
    gs = np.asarray(group_sizes)
    if not (gs.shape == (E,) and np.all(gs == T // E) and x.shape == (T, D)):
        return _kernel_np_fallback(np.asarray(x, np.float32), w1, b1, w3, b3,
                                   w2, b2, gs).astype(np.float32)

    in_maps = prep_in_maps(x, w1, b1, w3, b3, w2, b2)
    nc = _get_nc()
    res = run_bass_kernel_spmd(nc, in_maps, list(range(NCORES)))
    return np.concatenate([res.results[c]["out"] for c in range(NCORES)], axis=0)
